# revision 28
# baseline (speedup 1.0000x reference)
"""Trainium2 Bass kernel for nn_CalibratedNorm.

The reference module collapses algebraically to a per-(sample, channel)
affine:

    out[b,c,h,w] = x[b,c,h,w] * A[b,c] + S[b,c]

where, with gs/gsh the folded global-BN scale/shift and ms/msh the folded
mean-of-group-BNs scale/shift (all tiny [C] host math):

    alpha[b] = sigmoid( sum_c (alpha_w[c]/HW) * sum_hw x[b,c,:,:] + alpha_b )
    A[b,c]   = gs[c]  + alpha[b] * (ms[c]  - gs[c])
    S[b,c]   = gsh[c] + alpha[b] * (msh[c] - gsh[c])

Strategy: data-parallel over batch, 4 samples per core on 8 cores. The
kernel is HBM-streaming-bound, so x and out travel as bf16 (worst-case
~1% of max |out|, inside the 2e-2 gate with margin): 12.8 MB of HBM
traffic per core. The host pre-permutes each core's shard to [b, p, h, w]
(channel c = h*128 + p), so one partition's sample row is contiguous in
DRAM; sample 0's rows carry a 22 B parameter prefix (bf16 gate weights +
affine tables) so NO separate parameter DMA exists - params arrive with
the first bulk load, on the same queue. (A dedicated param DMA on any
other queue round-robins one tiny descriptor per bulk-packet turn and
lands ~12us late, stalling every alpha; even on the same queue its 256
tiny descriptors cost a ~0.7us HWDGE bubble before the bulk stream.)

Loads stream at half-sample (0.8 MB) granularity on the HWDGE SP ring;
ordering-only edges keep every load ahead of every store so alphas
resolve while loads stream. The gate dot z_b runs on the otherwise idle
PE: 14 accumulating chunk-matmuls (lhsT = bf16 wp column, rhs =
[128,448] x chunks) collapse the channel (partition) axis and 7x of the
free axis into one PSUM row [1,448]; ACT finishes with Copy+accum and
the sigmoid. This sidesteps the DVE/ACT accumulator row-sum paths,
which all run at 1x (~3.1-3.5us per half, measured); DVE tensor_reduce
likewise only has a 1x uop. DVE is left with the fused scale+shift
tensor_scalar ops, which hit the 4x bf16 mode (~1.1us per half), so
every engine sits far below the ~31us DMA ring floor and the ring never
starves: measured ring occupancy is ~97% with < 1us of gaps.
"""

import sys

import numpy as np

for _p in ("/opt/trn_rl_repo",):
    if _p not in sys.path:
        sys.path.insert(0, _p)

import ml_dtypes

import concourse.bacc as bacc
import concourse.bass as bass
import concourse.tile as tile
from concourse import mybir
from concourse.bass_utils import run_bass_kernel_spmd
from concourse.tile import add_dep_helper

EPS = 1e-5
B, C, H, W, G = 32, 256, 56, 56, 32
HW = H * W  # 3136
NCORES = 8
BPC = B // NCORES  # samples per core: 4
HALVES = C // 128  # channel partition-tiles per sample: 2
F32 = mybir.dt.float32
BF16 = mybir.dt.bfloat16
CH = 448  # gate-matmul chunk: 7 chunks x 448 = 3136, fits one PSUM bank
NCH = HW // CH
NPAR = 11  # bf16 prefix cols on sample 0: tab(4 x HALVES) | ab | wp(HALVES)
NPFX = 32  # prefix padded to 64 B so x rows stay 64B-aligned in DRAM
RW = NPFX + HALVES * HW  # padded DRAM row width per (b, p): 12608 B


def build_module() -> bass.Bass:
    # Bacc (not raw Bass): its compile() pass splits multi-sem waits into
    # EventSemaphore instructions — TRN2 allows at most 1 wait per
    # compute instruction and walrus codegen hard-errors otherwise.
    nc = bacc.Bacc("TRN2")

    x_in = nc.dram_tensor("x", [BPC * 128, RW], BF16, kind="ExternalInput")
    y_out = nc.dram_tensor("out", [BPC * 128, HALVES * HW], BF16, kind="ExternalOutput")

    with tile.TileContext(nc) as tc:
        with (
            tc.tile_pool(name="xp", bufs=BPC) as xp,
            tc.tile_pool(name="cs", bufs=1) as cs,
            tc.tile_pool(name="wk", bufs=2) as wk,
            tc.tile_pool(name="ps", bufs=2, space="PSUM") as ps,
        ):
            ones_row = cs.tile([1, 128], F32)
            nc.vector.memset(ones_row, 1.0)

            xv = x_in[:, :].rearrange("(b p) w -> b p w", p=128)
            yv = y_out[:, :].rearrange("(b p) w -> b p w", p=128)

            # Fully per-sample pipeline: sample b's store chases its own
            # load; no cross-sample barrier anywhere, so the DMA ring
            # never idles between the load phase and the store phase.
            loads = []
            stores = []
            tab = ab = wpb = None
            for b in range(BPC):
                # Tile holds [pfx | h0 row | h1 row] per partition; the x
                # halves sit at a NPFX-element offset.
                xt = xp.tile([128, RW], BF16, name=f"xt{b}", tag="xt")
                xh = [
                    xt[:, NPFX + h * HW : NPFX + (h + 1) * HW] for h in range(HALVES)
                ]
                zrow = ps.tile([1, CH], F32, name=f"zr{b}", tag="zr")
                # Half-sample (0.8MB) load granularity: half h's gate
                # matmuls run while half h^1 is still streaming in. The
                # b=0 h=0 load additionally carries the param prefix.
                for h in range(HALVES):
                    lo = 0 if (b == 0 and h == 0) else NPFX + h * HW
                    hi = NPFX + (h + 1) * HW
                    loads.append(
                        nc.sync.dma_start(out=xt[:, lo:hi], in_=xv[b][:, lo:hi])
                    )
                    if b == 0 and h == 0:
                        # Per-partition scalar operands must be fp32: one
                        # tiny DVE copy upcasts the tab/ab prefix columns.
                        tabf = cs.tile([128, 4 * HALVES + 1], F32)
                        nc.vector.tensor_copy(
                            out=tabf, in_=xt[:, 0 : 4 * HALVES + 1]
                        )
                        tab = tabf[:, 0 : 4 * HALVES].rearrange(
                            "p (f h) -> p f h", f=4
                        )
                        ab = tabf[0:1, 4 * HALVES : 4 * HALVES + 1]
                        wpb = xt[:, 4 * HALVES + 1 : NPAR]
                    # z_b accumulates on PE: the channel (partition) axis
                    # and 7x of the free axis collapse into one PSUM row.
                    for c in range(NCH):
                        nc.tensor.matmul(
                            zrow[:, :],
                            lhsT=wpb[:, h : h + 1],
                            rhs=xh[h][:, c * CH : (c + 1) * CH],
                            start=(h == 0 and c == 0),
                            stop=(h == HALVES - 1 and c == NCH - 1),
                        )
                # Finish the free axis on ACT (448 elems), then the gate.
                zscr = wk.tile([1, CH], F32, name=f"zs{b}", tag="zs")
                z = wk.tile([1, 1], F32, name=f"z{b}", tag="z")
                nc.scalar.activation(
                    out=zscr, in_=zrow[:, :],
                    func=mybir.ActivationFunctionType.Copy,
                    accum_out=z,
                )
                # alpha = sigmoid(z + alpha_b)
                al = wk.tile([1, 1], F32, name=f"al{b}", tag="al")
                nc.scalar.activation(
                    out=al, in_=z,
                    func=mybir.ActivationFunctionType.Sigmoid,
                    bias=ab, scale=1.0,
                )
                # broadcast alpha to all partitions, move to SBUF
                bc = ps.tile([128, 1], F32, name=f"bc{b}", tag="bc")
                nc.tensor.matmul(
                    bc[:, :], lhsT=ones_row[:, :], rhs=al[:, :],
                    start=True, stop=True,
                )
                ac = wk.tile([128, 1], F32, name=f"ac{b}", tag="ac")
                nc.vector.tensor_copy(out=ac, in_=bc[:, :])

                # A = gs + alpha*dms ; S = gsh + alpha*dmsh   [128,1] each
                A = wk.tile([128, HALVES], F32, name=f"A{b}", tag="A")
                Sh = wk.tile([128, HALVES], F32, name=f"S{b}", tag="S")
                for h in range(HALVES):
                    nc.vector.tensor_scalar(
                        out=A[:, h : h + 1], in0=tab[:, 1, h : h + 1],
                        scalar1=ac, scalar2=tab[:, 0, h : h + 1],
                        op0=mybir.AluOpType.mult, op1=mybir.AluOpType.add,
                    )
                    nc.vector.tensor_scalar(
                        out=Sh[:, h : h + 1], in0=tab[:, 3, h : h + 1],
                        scalar1=ac, scalar2=tab[:, 2, h : h + 1],
                        op0=mybir.AluOpType.mult, op1=mybir.AluOpType.add,
                    )

                # Fused affine on DVE (4x bf16 tensor_scalar), one op per
                # channel half; both halves are tile-adjacent, so the
                # sample stores as ONE contiguous [128, 12544B] DMA
                # (stores gate nothing - their data is ready at enqueue,
                # and 2x-bigger descriptors shave packet overhead).
                for h in range(HALVES):
                    nc.vector.tensor_scalar(
                        out=xh[h], in0=xh[h],
                        scalar1=A[:, h : h + 1], scalar2=Sh[:, h : h + 1],
                        op0=mybir.AluOpType.mult, op1=mybir.AluOpType.add,
                    )
                stores.append(
                    nc.sync.dma_start(
                        out=yv[b][:, :], in_=xt[:, NPFX : NPFX + HALVES * HW]
                    )
                )

            # Keep every load ahead of every store in the HWDGE ring:
            # ordering-only edges (no sems) from each store to the last
            # load. Without this the scheduler interleaves stores before
            # the last load, which delays the last alphas by ~10us.
            for st in stores:
                add_dep_helper(
                    st.ins, loads[-1].ins, sync=False,
                    reason="loads drain before stores on SP ring",
                )

    nc.compile()
    return nc


_NC_CACHE: list = []


def _get_module() -> bass.Bass:
    if not _NC_CACHE:
        _NC_CACHE.append(build_module())
    return _NC_CACHE[0]


def _prep_in_maps(inputs: dict) -> list[dict]:
    x = np.ascontiguousarray(np.asarray(inputs["x"], dtype=np.float32))
    alpha_w = np.asarray(inputs["alpha_w"], dtype=np.float32)
    alpha_b = np.asarray(inputs["alpha_b"], dtype=np.float32)
    g_w = np.asarray(inputs["g_w"], dtype=np.float32)
    g_b = np.asarray(inputs["g_b"], dtype=np.float32)
    g_rm = np.asarray(inputs["g_rm"], dtype=np.float32)
    g_rv = np.asarray(inputs["g_rv"], dtype=np.float32)
    grp_w = np.asarray(inputs["grp_w"], dtype=np.float32)
    grp_b = np.asarray(inputs["grp_b"], dtype=np.float32)
    grp_rm = np.asarray(inputs["grp_rm"], dtype=np.float32)
    grp_rv = np.asarray(inputs["grp_rv"], dtype=np.float32)

    gs = g_w / np.sqrt(g_rv + EPS)
    gsh = g_b - g_rm * gs
    sg = grp_w / np.sqrt(grp_rv + EPS)  # [G, C]
    ms = sg.mean(axis=0)
    msh = (grp_b - grp_rm * sg).mean(axis=0)
    dms = ms - gs
    dmsh = msh - gsh

    ch = (np.arange(HALVES)[None, :] * 128 + np.arange(128)[:, None])  # [128, HALVES]
    pfx = np.zeros((128, NPFX), dtype=np.float32)  # cols NPAR..NPFX stay 0 (pad)
    pfx[:, 0 * HALVES : 1 * HALVES] = gs[ch]
    pfx[:, 1 * HALVES : 2 * HALVES] = dms[ch]
    pfx[:, 2 * HALVES : 3 * HALVES] = gsh[ch]
    pfx[:, 3 * HALVES : 4 * HALVES] = dmsh[ch]
    pfx[0, 4 * HALVES] = alpha_b.reshape(-1)[0]
    pfx[:, 4 * HALVES + 1 : NPAR] = alpha_w[ch] / np.float32(HW)

    # Permute each core's shard to [b, p, h, w] so both channel halves of
    # a partition are DRAM-contiguous, then prepend the param prefix to
    # every (b, p) row (only b=0's copy is read on device).
    xb = np.ascontiguousarray(
        x.reshape(NCORES, BPC, HALVES, 128, HW).transpose(0, 1, 3, 2, 4)
    ).reshape(NCORES, BPC * 128, HALVES * HW)
    full = np.empty((NCORES, BPC * 128, RW), dtype=ml_dtypes.bfloat16)
    full[:, :, NPFX:] = xb
    full[:, :, :NPFX] = np.tile(pfx.astype(ml_dtypes.bfloat16), (BPC, 1))
    in_maps = []
    for k in range(NCORES):
        in_maps.append({"x": full[k]})
    return in_maps


def _unpermute_core(y: np.ndarray) -> np.ndarray:
    """Inverse of the host-side [b, p, h, w] shard permutation."""
    return (
        y.astype(np.float32)
        .reshape(BPC, 128, HALVES, HW)
        .transpose(0, 2, 1, 3)
        .reshape(BPC, C, H, W)
    )


def _run(inputs: dict, trace: bool = False, trace_cores=None):
    nc = _get_module()
    in_maps = _prep_in_maps(inputs)
    res = run_bass_kernel_spmd(
        nc, in_maps, core_ids=list(range(NCORES)), trace=trace,
        trace_cores=trace_cores,
    )
    outs = [_unpermute_core(np.asarray(r["out"])) for r in res.results]
    full = np.concatenate(outs, axis=0)
    return full, res


def kernel(**inputs) -> np.ndarray:
    out, _ = _run(inputs, trace=False)
    return out


# revision 29
# speedup vs baseline: 1.1301x; 1.1301x over previous
"""Trainium2 Bass kernel for nn_CalibratedNorm.

The reference module collapses algebraically to a per-(sample, channel)
affine:

    out[b,c,h,w] = x[b,c,h,w] * A[b,c] + S[b,c]

where, with gs/gsh the folded global-BN scale/shift and ms/msh the folded
mean-of-group-BNs scale/shift (all tiny [C] host math):

    alpha[b] = sigmoid( sum_c (alpha_w[c]/HW) * sum_hw x[b,c,:,:] + alpha_b )
    A[b,c]   = gs[c]  + alpha[b] * (ms[c]  - gs[c])
    S[b,c]   = gsh[c] + alpha[b] * (msh[c] - gsh[c])

Strategy: data-parallel over batch, 4 samples per core on 8 cores. The
kernel is HBM-streaming-bound, so x and out travel as bf16 (worst-case
~1% of max |out|, inside the 2e-2 gate with margin): 12.8 MB of HBM
traffic per core. The host pre-permutes each core's shard to [b, p, h, w]
(channel c = h*128 + p), so one partition's sample row is contiguous in
DRAM; sample 0's rows carry a 22 B parameter prefix (bf16 gate weights +
affine tables) so NO separate parameter DMA exists - params arrive with
the first bulk load, on the same queue. (A dedicated param DMA on any
other queue round-robins one tiny descriptor per bulk-packet turn and
lands ~12us late, stalling every alpha; even on the same queue its 256
tiny descriptors cost a ~0.7us HWDGE bubble before the bulk stream.)

Loads stream at half-sample (0.8 MB) granularity on the HWDGE SP ring;
ordering-only edges keep every load ahead of every store so alphas
resolve while loads stream. The gate dot z_b runs on the otherwise idle
PE: 14 accumulating chunk-matmuls (lhsT = bf16 wp column, rhs =
[128,448] x chunks) collapse the channel (partition) axis and 7x of the
free axis into one PSUM row [1,448]; ACT finishes with Copy+accum and
the sigmoid. This sidesteps the DVE/ACT accumulator row-sum paths,
which all run at 1x (~3.1-3.5us per half, measured); DVE tensor_reduce
likewise only has a 1x uop. DVE is left with the fused scale+shift
tensor_scalar ops, which hit the 4x bf16 mode (~1.1us per half), so
every engine sits far below the ~31us DMA ring floor and the ring never
starves: measured ring occupancy is ~97% with < 1us of gaps.
"""

import sys

import numpy as np

for _p in ("/opt/trn_rl_repo",):
    if _p not in sys.path:
        sys.path.insert(0, _p)

import ml_dtypes

import concourse.bacc as bacc
import concourse.bass as bass
import concourse.tile as tile
from concourse import mybir
from concourse.bass_utils import run_bass_kernel_spmd
from concourse.tile import add_dep_helper

EPS = 1e-5
B, C, H, W, G = 32, 256, 56, 56, 32
HW = H * W  # 3136
NCORES = 8
BPC = B // NCORES  # samples per core: 4
HALVES = C // 128  # channel partition-tiles per sample: 2
F32 = mybir.dt.float32
BF16 = mybir.dt.bfloat16
CH = 448  # gate-matmul chunk: 7 chunks x 448 = 3136, fits one PSUM bank
NCH = HW // CH
NPAR = 11  # bf16 prefix cols on sample 0: tab(4 x HALVES) | ab | wp(HALVES)
NPFX = 32  # prefix padded to 64 B so x rows stay 64B-aligned in DRAM
RW = NPFX + HALVES * HW  # padded DRAM row width per (b, p): 12608 B


def build_module() -> bass.Bass:
    # Bacc (not raw Bass): its compile() pass splits multi-sem waits into
    # EventSemaphore instructions — TRN2 allows at most 1 wait per
    # compute instruction and walrus codegen hard-errors otherwise.
    nc = bacc.Bacc("TRN2")

    x_in = nc.dram_tensor("x", [BPC * 128, RW], BF16, kind="ExternalInput")
    y_out = nc.dram_tensor("out", [BPC * 128, HALVES * HW], BF16, kind="ExternalOutput")

    with tile.TileContext(nc) as tc:
        with (
            tc.tile_pool(name="xp", bufs=BPC) as xp,
            tc.tile_pool(name="cs", bufs=1) as cs,
            tc.tile_pool(name="wk", bufs=2) as wk,
            tc.tile_pool(name="ps", bufs=2, space="PSUM") as ps,
        ):
            ones_row = cs.tile([1, 128], F32)
            nc.vector.memset(ones_row, 1.0)

            xv = x_in[:, :].rearrange("(b p) w -> b p w", p=128)
            yv = y_out[:, :].rearrange("(b p) w -> b p w", p=128)

            # Fully per-sample pipeline: sample b's store chases its own
            # load; no cross-sample barrier anywhere, so the DMA ring
            # never idles between the load phase and the store phase.
            loads = []
            stores = []
            tab = ab = wpb = None
            for b in range(BPC):
                # Tile holds [pfx | h0 row | h1 row] per partition; the x
                # halves sit at a NPFX-element offset.
                xt = xp.tile([128, RW], BF16, name=f"xt{b}", tag="xt")
                xh = [
                    xt[:, NPFX + h * HW : NPFX + (h + 1) * HW] for h in range(HALVES)
                ]
                zrow = ps.tile([1, CH], F32, name=f"zr{b}", tag="zr")
                # One full-sample load (1.6MB, 12.5KB/partition descriptors
                # to amortize per-packet overhead; alphas still resolve
                # with ~7us of slack before the ring wants the stores).
                # The b=0 load additionally carries the param prefix.
                lo = 0 if b == 0 else NPFX
                loads.append(
                    nc.sync.dma_start(
                        out=xt[:, lo : NPFX + HALVES * HW],
                        in_=xv[b][:, lo : NPFX + HALVES * HW],
                    )
                )
                for h in range(HALVES):
                    if b == 0 and h == 0:
                        # Per-partition scalar operands must be fp32: one
                        # tiny DVE copy upcasts the tab/ab prefix columns.
                        tabf = cs.tile([128, 4 * HALVES + 1], F32)
                        nc.vector.tensor_copy(
                            out=tabf, in_=xt[:, 0 : 4 * HALVES + 1]
                        )
                        tab = tabf[:, 0 : 4 * HALVES].rearrange(
                            "p (f h) -> p f h", f=4
                        )
                        ab = tabf[0:1, 4 * HALVES : 4 * HALVES + 1]
                        wpb = xt[:, 4 * HALVES + 1 : NPAR]
                    # z_b accumulates on PE: the channel (partition) axis
                    # and 7x of the free axis collapse into one PSUM row.
                    for c in range(NCH):
                        nc.tensor.matmul(
                            zrow[:, :],
                            lhsT=wpb[:, h : h + 1],
                            rhs=xh[h][:, c * CH : (c + 1) * CH],
                            start=(h == 0 and c == 0),
                            stop=(h == HALVES - 1 and c == NCH - 1),
                        )
                # Finish the free axis on ACT (448 elems), then the gate.
                zscr = wk.tile([1, CH], F32, name=f"zs{b}", tag="zs")
                z = wk.tile([1, 1], F32, name=f"z{b}", tag="z")
                nc.scalar.activation(
                    out=zscr, in_=zrow[:, :],
                    func=mybir.ActivationFunctionType.Copy,
                    accum_out=z,
                )
                # alpha = sigmoid(z + alpha_b)
                al = wk.tile([1, 1], F32, name=f"al{b}", tag="al")
                nc.scalar.activation(
                    out=al, in_=z,
                    func=mybir.ActivationFunctionType.Sigmoid,
                    bias=ab, scale=1.0,
                )
                # broadcast alpha to all partitions, move to SBUF
                bc = ps.tile([128, 1], F32, name=f"bc{b}", tag="bc")
                nc.tensor.matmul(
                    bc[:, :], lhsT=ones_row[:, :], rhs=al[:, :],
                    start=True, stop=True,
                )
                ac = wk.tile([128, 1], F32, name=f"ac{b}", tag="ac")
                nc.vector.tensor_copy(out=ac, in_=bc[:, :])

                # A = gs + alpha*dms ; S = gsh + alpha*dmsh   [128,1] each
                A = wk.tile([128, HALVES], F32, name=f"A{b}", tag="A")
                Sh = wk.tile([128, HALVES], F32, name=f"S{b}", tag="S")
                for h in range(HALVES):
                    nc.vector.tensor_scalar(
                        out=A[:, h : h + 1], in0=tab[:, 1, h : h + 1],
                        scalar1=ac, scalar2=tab[:, 0, h : h + 1],
                        op0=mybir.AluOpType.mult, op1=mybir.AluOpType.add,
                    )
                    nc.vector.tensor_scalar(
                        out=Sh[:, h : h + 1], in0=tab[:, 3, h : h + 1],
                        scalar1=ac, scalar2=tab[:, 2, h : h + 1],
                        op0=mybir.AluOpType.mult, op1=mybir.AluOpType.add,
                    )

                # Fused affine on DVE (4x bf16 tensor_scalar), one op per
                # channel half; both halves are tile-adjacent, so the
                # sample stores as ONE contiguous [128, 12544B] DMA
                # (stores gate nothing - their data is ready at enqueue,
                # and 2x-bigger descriptors shave packet overhead).
                for h in range(HALVES):
                    nc.vector.tensor_scalar(
                        out=xh[h], in0=xh[h],
                        scalar1=A[:, h : h + 1], scalar2=Sh[:, h : h + 1],
                        op0=mybir.AluOpType.mult, op1=mybir.AluOpType.add,
                    )
                stores.append(
                    nc.sync.dma_start(
                        out=yv[b][:, :], in_=xt[:, NPFX : NPFX + HALVES * HW]
                    )
                )

            # Keep every load ahead of every store in the HWDGE ring:
            # ordering-only edges (no sems) from each store to the last
            # load. Without this the scheduler interleaves stores before
            # the last load, which delays the last alphas by ~10us.
            for st in stores:
                add_dep_helper(
                    st.ins, loads[-1].ins, sync=False,
                    reason="loads drain before stores on SP ring",
                )

    nc.compile()
    return nc


_NC_CACHE: list = []


def _get_module() -> bass.Bass:
    if not _NC_CACHE:
        _NC_CACHE.append(build_module())
    return _NC_CACHE[0]


def _prep_in_maps(inputs: dict) -> list[dict]:
    x = np.ascontiguousarray(np.asarray(inputs["x"], dtype=np.float32))
    alpha_w = np.asarray(inputs["alpha_w"], dtype=np.float32)
    alpha_b = np.asarray(inputs["alpha_b"], dtype=np.float32)
    g_w = np.asarray(inputs["g_w"], dtype=np.float32)
    g_b = np.asarray(inputs["g_b"], dtype=np.float32)
    g_rm = np.asarray(inputs["g_rm"], dtype=np.float32)
    g_rv = np.asarray(inputs["g_rv"], dtype=np.float32)
    grp_w = np.asarray(inputs["grp_w"], dtype=np.float32)
    grp_b = np.asarray(inputs["grp_b"], dtype=np.float32)
    grp_rm = np.asarray(inputs["grp_rm"], dtype=np.float32)
    grp_rv = np.asarray(inputs["grp_rv"], dtype=np.float32)

    gs = g_w / np.sqrt(g_rv + EPS)
    gsh = g_b - g_rm * gs
    sg = grp_w / np.sqrt(grp_rv + EPS)  # [G, C]
    ms = sg.mean(axis=0)
    msh = (grp_b - grp_rm * sg).mean(axis=0)
    dms = ms - gs
    dmsh = msh - gsh

    ch = (np.arange(HALVES)[None, :] * 128 + np.arange(128)[:, None])  # [128, HALVES]
    pfx = np.zeros((128, NPFX), dtype=np.float32)  # cols NPAR..NPFX stay 0 (pad)
    pfx[:, 0 * HALVES : 1 * HALVES] = gs[ch]
    pfx[:, 1 * HALVES : 2 * HALVES] = dms[ch]
    pfx[:, 2 * HALVES : 3 * HALVES] = gsh[ch]
    pfx[:, 3 * HALVES : 4 * HALVES] = dmsh[ch]
    pfx[0, 4 * HALVES] = alpha_b.reshape(-1)[0]
    pfx[:, 4 * HALVES + 1 : NPAR] = alpha_w[ch] / np.float32(HW)

    # Permute each core's shard to [b, p, h, w] so both channel halves of
    # a partition are DRAM-contiguous, then prepend the param prefix to
    # every (b, p) row (only b=0's copy is read on device).
    xb = np.ascontiguousarray(
        x.reshape(NCORES, BPC, HALVES, 128, HW).transpose(0, 1, 3, 2, 4)
    ).reshape(NCORES, BPC * 128, HALVES * HW)
    full = np.empty((NCORES, BPC * 128, RW), dtype=ml_dtypes.bfloat16)
    full[:, :, NPFX:] = xb
    full[:, :, :NPFX] = np.tile(pfx.astype(ml_dtypes.bfloat16), (BPC, 1))
    in_maps = []
    for k in range(NCORES):
        in_maps.append({"x": full[k]})
    return in_maps


def _unpermute_core(y: np.ndarray) -> np.ndarray:
    """Inverse of the host-side [b, p, h, w] shard permutation."""
    return (
        y.astype(np.float32)
        .reshape(BPC, 128, HALVES, HW)
        .transpose(0, 2, 1, 3)
        .reshape(BPC, C, H, W)
    )


def _run(inputs: dict, trace: bool = False, trace_cores=None):
    nc = _get_module()
    in_maps = _prep_in_maps(inputs)
    res = run_bass_kernel_spmd(
        nc, in_maps, core_ids=list(range(NCORES)), trace=trace,
        trace_cores=trace_cores,
    )
    outs = [_unpermute_core(np.asarray(r["out"])) for r in res.results]
    full = np.concatenate(outs, axis=0)
    return full, res


def kernel(**inputs) -> np.ndarray:
    out, _ = _run(inputs, trace=False)
    return out
